# revision 1
# baseline (speedup 1.0000x reference)
"""Two-layer GCN block (PyG GCNConv x2, no nonlinearity) on 8 trn2 NeuronCores.

Math: out1 = D^-1/2 (A+I) D^-1/2 (x W1) + b1 ; out2 = same on out1 with W2, b2.
Factorization used on device:
    u  = dis (.) x                     (row scale, dis = deg^-1/2)
    A[d] = sum_{e: src->d} u[src]      (plain segment sum incl. self loops)
    out1 = dis (.) (A @ W1) + b1
    v    = dis (.) out1                (gather source for layer 2)
    out2 = dis (.) (A2 @ W2) + b2
Sharding: destinations split across 8 cores (6250 each, padded to 6272 = 49*128).
Each core gathers u rows by source id from a replicated (AllGather'd) table,
accumulates per 128-dest tile in PSUM via identity matmuls, applies the dense
64x64 weight per tile, and scatters rows back. Host does index prep only.
"""
import sys
import numpy as np

sys.path.insert(0, '/root/.axon_site')
sys.path.insert(0, '/opt/trn_rl_repo')

N = 50000
E = 800000
D = 64
C = 8              # cores
NSH = 6250         # real dests per core
NT = 49            # dest tiles per core
NSHP = NT * 128    # 6272 padded dests per core
NFULL = C * NSHP   # 50176
P = 128
GZROW = 6250       # padded-global id of an all-zero row (shard 0 pad row)

_compiled = None   # (nc, S, D_list) cache across calls


def _gpad(src):
    """original node id -> padded-global row id"""
    return (src // NSH) * NSHP + (src % NSH)


def _build_schedule(col_sorted_by_core, srcs_by_core):
    """Common per-tile slot counts across cores + per-core index arrays."""
    # degrees per core (local dest id -> degree incl self loop)
    degs = []
    perms = []
    for k in range(C):
        deg = np.bincount(col_sorted_by_core[k], minlength=NSH) + 1
        perm = np.argsort(-deg, kind='stable')
        degs.append(deg)
        perms.append(perm)
    D_list = []
    for t in range(NT):
        lo, hi = t * 128, min((t + 1) * 128, NSH)
        dmax = 0
        for k in range(C):
            dmax = max(dmax, int(degs[k][perms[k][lo:hi]].max()) - 1)
        D_list.append(dmax)
    return degs, perms, D_list


def kernel(x, edge_index, W1, b1, W2, b2):
    import concourse.bass as bass
    import concourse.bacc as bacc
    import concourse.mybir as mybir
    from concourse import tile
    from concourse.bass_utils import run_bass_kernel_spmd

    x = np.asarray(x, dtype=np.float32)
    edge_index = np.asarray(edge_index)
    W1 = np.asarray(W1, dtype=np.float32)
    W2 = np.asarray(W2, dtype=np.float32)
    b1 = np.asarray(b1, dtype=np.float32)
    b2 = np.asarray(b2, dtype=np.float32)

    row = edge_index[0].astype(np.int64)   # sources
    col = edge_index[1].astype(np.int64)   # destinations

    deg = np.bincount(col, minlength=N).astype(np.float32) + 1.0  # + self loop
    dis = (1.0 / np.sqrt(deg)).astype(np.float32)

    # ---- per-core edge lists (dest-sharded) ----
    core_of = col // NSH
    order = np.argsort(col, kind='stable')
    col_s, row_s = col[order], row[order]
    # boundaries between cores in the dest-sorted edge list
    bounds = np.searchsorted(col_s, np.arange(0, N + 1, NSH))
    col_by_core, src_by_core = [], []
    for k in range(C):
        sl = slice(bounds[k], bounds[k + 1])
        col_by_core.append((col_s[sl] - k * NSH).astype(np.int64))
        src_by_core.append(row_s[sl])

    degs, perms, D_list = _build_schedule(col_by_core, src_by_core)
    S = int(np.sum(D_list))
    offs = np.concatenate([[0], np.cumsum(D_list)]).astype(np.int64)

    # ---- per-core host arrays ----
    idx_arr = np.full((C, P, S), GZROW, dtype=np.int32)
    idx2_arr = np.full((C, P, S), 47466, dtype=np.int32)
    disrc_arr = np.zeros((C, P, S), dtype=np.float32)
    disd_arr = np.zeros((C, P, NT), dtype=np.float32)
    disd2_arr = np.zeros((C, P, NT), dtype=np.float32)
    disd_rows_arr = np.zeros((C, 1, NSHP), dtype=np.float32)
    # replicated padded x table (zero pad rows) in padded-global coords
    x_pad = np.zeros((NFULL, D), dtype=np.float32)
    for k in range(C):
        x_pad[k * NSHP:k * NSHP + NSH] = x[k * NSH:(k + 1) * NSH]
    # own-shard x rows in degree-sorted order (self-loop block per tile)
    x_own_arr = np.zeros((C, NSHP, D), dtype=np.float32)

    # perm-inverse per core: orig local dest id -> sorted row position.
    # Output tiles write contiguously in sorted order (plain DMA, not
    # indirect scatter); layer-2 gather indices absorb the permutation.
    pinvs = []
    for k in range(C):
        pinv = np.empty(NSH, dtype=np.int64)
        pinv[perms[k]] = np.arange(NSH)
        pinvs.append(pinv)

    # AllGather pieces (tile-aligned): layer-1 output is gathered in 4
    # chunks so the first 3 overlap the gather stream. vfull layout is
    # piece-major then rank-major.
    TB = [0, 18, 33, 46, 49]                    # tile boundaries (front-loaded
    # so the last AllGather piece is small and barely exposed)
    piece_lo = np.array([0, 2304, 4224, 5888])
    piece_rows = np.array([2304, 1920, 1664, 384])
    region_base = np.array([0, 18432, 33792, 47104])
    pin_all = np.concatenate(pinvs)             # orig id -> permuted position
    korig = np.arange(N) // NSH
    pc = np.digitize(pin_all, piece_lo[1:])     # piece index of each position
    g2_of = (region_base[pc] + korig * piece_rows[pc]
             + (pin_all - piece_lo[pc]))        # orig id -> vfull row

    for k in range(C):
        lc, ls = col_by_core[k], src_by_core[k]
        # CSR by local dest: edges sorted by local dest already (stable sort)
        starts = np.searchsorted(lc, np.arange(NSH + 1))
        perm = perms[k]
        dval = dis[k * NSH + perm]                      # dis of sorted dests
        for t in range(NT):
            lo = t * 128
            nreal = min(128, NSH - lo)
            pv = perm[lo:lo + nreal]                    # local dest ids, sorted pos
            disd_arr[k, :nreal, t] = dval[lo:lo + nreal]
            # slots
            o = offs[t]
            for j, v in enumerate(pv):
                e0, e1 = starts[v], starts[v + 1]
                nsrc = e1 - e0
                idx_arr[k, j, o:o + nsrc] = _gpad(ls[e0:e1])
                idx2_arr[k, j, o:o + nsrc] = g2_of[ls[e0:e1]]
                disrc_arr[k, j, o:o + nsrc] = dis[ls[e0:e1]]
        disd2_arr[k] = disd_arr[k] ** 2
        x_own_arr[k, :NSH] = x[k * NSH + perm]
        # disd by sorted row position (for the bias outer product, layer 1)
        dr = np.zeros(NSHP, dtype=np.float32)
        dr[:NSH] = dis[k * NSH + perm]
        disd_rows_arr[k, 0, :] = dr

    # sanity: each dest's slot count fits (degree incl self loop <= D_t)
    # (guaranteed by schedule construction)

    # ---- build device program ----
    nc = bacc.Bacc(None, target_bir_lowering=False)
    dt = mybir.dt
    xtab = nc.declare_dram_parameter("xtab", [NFULL, D], dt.float32, isOutput=False)
    identp = nc.declare_dram_parameter("identp", [P, P], dt.float32, isOutput=False)
    xown = nc.declare_dram_parameter("xown", [NSHP, D], dt.float32, isOutput=False)
    dsrc = nc.declare_dram_parameter("dsrc", [P, S], dt.float32, isOutput=False)
    idxp = nc.declare_dram_parameter("idxp", [P, S], dt.int32, isOutput=False)
    idxp2 = nc.declare_dram_parameter("idxp2", [P, S], dt.int32, isOutput=False)
    disd = nc.declare_dram_parameter("disd", [P, NT], dt.float32, isOutput=False)
    disd2 = nc.declare_dram_parameter("disd2", [P, NT], dt.float32, isOutput=False)
    disdr = nc.declare_dram_parameter("disdr", [1, NSHP], dt.float32, isOutput=False)
    w1p = nc.declare_dram_parameter("w1p", [D, D], dt.float32, isOutput=False)
    w2p = nc.declare_dram_parameter("w2p", [D, D], dt.float32, isOutput=False)
    b1p = nc.declare_dram_parameter("b1p", [1, D], dt.float32, isOutput=False)
    b2p = nc.declare_dram_parameter("b2p", [1, D], dt.float32, isOutput=False)
    out_sh = nc.declare_dram_parameter("out_sh", [NSHP, D], dt.float32, isOutput=True)

    v_shp = [nc.dram_tensor(f"v_sh{i}", [int(piece_rows[i]), D], dt.float32)
             for i in range(4)]
    vfull = nc.dram_tensor("vfull", [NFULL, D], dt.float32, addr_space="Shared")

    rg = [list(range(C))]

    with tile.TileContext(nc) as tc:
        with tc.tile_pool(name="const", bufs=1) as cp, \
             tc.tile_pool(name="sb", bufs=4) as pool, \
             tc.tile_pool(name="gp", bufs=3) as gpool, \
             tc.tile_pool(name="ep", bufs=3) as ep, \
             tc.tile_pool(name="psA", bufs=2, space="PSUM") as psA, \
             tc.tile_pool(name="psB", bufs=2, space="PSUM") as psB, \
             tc.tile_pool(name="psC", bufs=2, space="PSUM") as psC:

            ident = cp.tile([P, P], dt.float32)
            nc.sync.dma_start(out=ident[:], in_=identp[:, :])
            w1t = cp.tile([D, D], dt.float32)
            nc.sync.dma_start(out=w1t[:], in_=w1p[:, :])
            w2t = cp.tile([D, D], dt.float32)
            nc.sync.dma_start(out=w2t[:], in_=w2p[:, :])
            b1t = cp.tile([1, D], dt.float32)
            nc.sync.dma_start(out=b1t[:], in_=b1p[:, :])
            b2t = cp.tile([1, D], dt.float32)
            nc.sync.dma_start(out=b2t[:], in_=b2p[:, :])
            it = cp.tile([P, S], dt.int32)
            h = int(offs[1])
            nc.sync.dma_start(out=it[:, :h], in_=idxp[:, :h])
            nc.sync.dma_start(out=it[:, h:], in_=idxp[:, h:])
            it2 = cp.tile([P, S], dt.int32)
            nc.sync.dma_start(out=it2[:], in_=idxp2[:, :])
            dst_ = cp.tile([P, S], dt.float32)
            nc.sync.dma_start(out=dst_[:], in_=dsrc[:, :])
            dd1 = cp.tile([P, NT], dt.float32)
            nc.sync.dma_start(out=dd1[:], in_=disd[:, :])
            dd2 = cp.tile([P, NT], dt.float32)
            nc.sync.dma_start(out=dd2[:], in_=disd2[:, :])
            ddr = cp.tile([1, NSHP], dt.float32)
            nc.sync.dma_start(out=ddr[:], in_=disdr[:, :])
            ones_row = cp.tile([1, P], dt.float32)
            nc.vector.memset(ones_row[:], 1.0)

            def layer(src_full, idx_tile, wtile, btile, bias_rhs, scale_tile,
                      dest_of, self_of, slot_scale=None, post_tile=None):
                maxD = max(D_list)
                for t in range(NT):
                    acc = psA.tile([P, D], dt.float32)
                    dcount = D_list[t]
                    # one buffer per dest-tile: slice writes share a tile
                    # generation, so only the first DMA needs a slot wait
                    gbuf = gpool.tile([P, maxD * D], dt.float32, tag="g")
                    for s in range(dcount):
                        nc.gpsimd.indirect_dma_start(
                            out=gbuf[:, s * D:(s + 1) * D], out_offset=None,
                            in_=src_full[:],
                            in_offset=bass.IndirectOffsetOnAxis(
                                ap=idx_tile[:, int(offs[t]) + s:
                                            int(offs[t]) + s + 1],
                                axis=0))
                    if slot_scale is not None:
                        gsb = gpool.tile([P, maxD * D], dt.float32, tag="gs")
                        for s in range(dcount):
                            nc.scalar.activation(
                                out=gsb[:, s * D:(s + 1) * D],
                                in_=gbuf[:, s * D:(s + 1) * D],
                                func=mybir.ActivationFunctionType.Copy,
                                scale=slot_scale[:, int(offs[t]) + s:
                                                 int(offs[t]) + s + 1])
                        rsrc = gsb
                    else:
                        rsrc = gbuf
                    # self-loop block: contiguous rows, loaded off-queue
                    stens, soff = self_of(t)
                    st = ep.tile([P, D], dt.float32, tag="sl")
                    nc.sync.dma_start(out=st[:], in_=stens[soff:soff + P, :])
                    if slot_scale is not None:
                        st2 = ep.tile([P, D], dt.float32, tag="sl2")
                        nc.scalar.activation(
                            out=st2[:], in_=st[:],
                            func=mybir.ActivationFunctionType.Copy,
                            scale=scale_self[:, t:t + 1])
                        st = st2
                    nc.tensor.matmul(acc[:], lhsT=ident[:], rhs=st[:],
                                     start=True, stop=(dcount == 0))
                    for s in range(dcount):
                        nc.tensor.matmul(acc[:], lhsT=ident[:],
                                         rhs=rsrc[:, s * D:(s + 1) * D],
                                         start=False, stop=(s == dcount - 1))
                    # scale rows (dests on partitions)
                    csb = ep.tile([P, D], dt.float32, tag="c")
                    nc.scalar.activation(
                        out=csb[:], in_=acc[:],
                        func=mybir.ActivationFunctionType.Copy,
                        scale=scale_tile[:, t:t + 1])
                    # transpose -> [64, 128]
                    tr1 = psB.tile([D, P], dt.float32)
                    nc.tensor.transpose(tr1[:], csb[:], ident[:])
                    ct = ep.tile([D, P], dt.float32, tag="ct")
                    nc.vector.tensor_copy(out=ct[:], in_=tr1[:])
                    # W^T @ C^T (+ bias outer)
                    pv = psC.tile([D, P], dt.float32)
                    nc.tensor.matmul(pv[:], lhsT=wtile[:], rhs=ct[:],
                                     start=True, stop=False)
                    nc.tensor.matmul(pv[:], lhsT=btile[:], rhs=bias_rhs(t),
                                     start=False, stop=True)
                    vt = ep.tile([D, P], dt.float32, tag="vt")
                    nc.vector.tensor_copy(out=vt[:], in_=pv[:])
                    # transpose back -> [128, 64]
                    tr2 = psB.tile([P, D], dt.float32)
                    nc.tensor.matmul(tr2[:], lhsT=vt[:], rhs=ident[:D, :D],
                                     is_transpose=True)
                    vsb = ep.tile([P, D], dt.float32, tag="vs")
                    nc.vector.tensor_copy(out=vsb[:], in_=tr2[:])
                    # rows are in degree-sorted order -> contiguous write
                    # (HWDGE; keeps the GpSimd queue free for gathers)
                    dtens, doff = dest_of(t)
                    nc.sync.dma_start(out=dtens[doff:doff + P, :], in_=vsb[:])
                    if post_tile is not None:
                        post_tile(t)

            # layer 1 -> v_sh (= dis (.) out1, degree-sorted row order);
            # gathers raw x, scales by dis[src] per slot on the Scalar engine
            def v_dest(t):
                i = 0
                while t >= TB[i + 1]:
                    i += 1
                return v_shp[i], (t - TB[i]) * P

            def fire_ag(t):
                for i in range(4):
                    if t == TB[i + 1] - 1:
                        lo = int(region_base[i])
                        hi = lo + 8 * int(piece_rows[i])
                        nc.gpsimd.collective_compute(
                            "AllGather", mybir.AluOpType.bypass,
                            replica_groups=rg,
                            ins=[v_shp[i][:]], outs=[vfull[lo:hi, :]])

            scale_self = dd1   # dis[dest] for the layer-1 self block
            layer(xtab, it, w1t, b1t,
                  lambda t: ddr[:, t * P:(t + 1) * P],
                  dd2, v_dest, lambda t: (xown, t * P),
                  slot_scale=dst_, post_tile=fire_ag)
            # layer 2 -> out_sh (degree-sorted row order; host un-permutes)
            layer(vfull, it2, w2t, b2t,
                  lambda t: ones_row[:, :],
                  dd1, lambda t: (out_sh, t * P), v_dest)

    nc.compile()

    in_maps = []
    for k in range(C):
        in_maps.append({
            "xtab": x_pad, "identp": np.eye(P, dtype=np.float32),
            "xown": x_own_arr[k],
            "dsrc": disrc_arr[k],
            "idxp": idx_arr[k], "idxp2": idx2_arr[k],
            "disd": disd_arr[k], "disd2": disd2_arr[k],
            "disdr": disd_rows_arr[k],
            "w1p": W1, "w2p": W2,
            "b1p": b1.reshape(1, D), "b2p": b2.reshape(1, D),
        })
    global _compiled
    _compiled = (nc, in_maps)
    res = run_bass_kernel_spmd(nc, in_maps, list(range(C)))
    out = np.empty((N, D), dtype=np.float32)
    for k in range(C):
        # rows come back in degree-sorted order; un-permute
        out[k * NSH + perms[k]] = res.results[k]["out_sh"][:NSH]
    return out


def profile_last():
    """Re-run the last compiled program with NTFF tracing; returns exec ns."""
    from concourse.bass_utils import run_bass_kernel_spmd
    assert _compiled is not None
    nc, in_maps = _compiled
    r = run_bass_kernel_spmd(nc, in_maps, list(range(C)), trace=True)
    return r.exec_time_ns



# revision 9
# speedup vs baseline: 1.6315x; 1.6315x over previous
"""Two-layer GCN block (PyG GCNConv x2) on 8 trn2 NeuronCores.

Math per layer: out = D^-1/2 (A+I) D^-1/2 (h W) + b, done as
    acc[d] = sum_m scale_m * tab[src_m]   (messages incl. self loops,
                                           scale = dis_s*dis_d, self dis_d^2)
    out    = acc @ W + b
Device mapping: dests dealt degree-round-robin into 392 bins of 128
(-> 8 cores x 49 tile slots, common block schedule across cores). Per
tile the messages are bulk-gathered with gpsimd.dma_gather (two calls
per 7-tile group, one per table half for the int16 index range), then
accumulated in PSUM via matmuls acc += Sel.T @ G where Sel is built on
the Vector engine as (iota == destrow) * scale. Layer-1 outputs are
AllGathered in 4 pieces into a shared vfull table for layer 2.
"""
import numpy as np

import sys
sys.path.insert(0, '/root/.axon_site')
sys.path.insert(0, '/opt/trn_rl_repo')

N = 50000
D = 64
C = 8
NT = 49                      # dest tile slots per core
NBINS = C * NT               # 392
NFULL = NBINS * 128          # 50176
P = 128
GROUP = 7                    # tiles per gather group
TB = [0, 13, 25, 37, 49]     # AllGather piece boundaries (tile slots)
XSPLIT = 32768               # layer-1 gather table split
VSPLIT_PIECE = 2             # layer-2 split: pieces [0,1] vs [2,3]

PIECE_ROWS = [(TB[i + 1] - TB[i]) * 128 for i in range(4)]          # per core
REGION = np.concatenate([[0], np.cumsum([C * r for r in PIECE_ROWS])])
VSPLIT = int(REGION[VSPLIT_PIECE])

_compiled = None


def _build(edge_index):
    row = np.asarray(edge_index[0], dtype=np.int64)
    col = np.asarray(edge_index[1], dtype=np.int64)

    deg = np.bincount(col, minlength=N).astype(np.float32) + 1.0
    dis = (1.0 / np.sqrt(deg)).astype(np.float32)

    # dest -> (core, slot, row) by degree-balanced round-robin deal
    order = np.argsort(-deg, kind='stable')
    binof = np.empty(N, dtype=np.int64)
    rowof = np.empty(N, dtype=np.int64)
    binof[order] = np.arange(N) % NBINS
    rowof[order] = np.arange(N) // NBINS
    coreof = binof % C
    slotof = binof // C

    pieceof = np.digitize(slotof, TB[1:])
    pr = np.array(PIECE_ROWS, dtype=np.int64)
    tbl = np.array(TB[:4], dtype=np.int64)
    vrow = (REGION[pieceof] + coreof * pr[pieceof]
            + (slotof - tbl[pieceof]) * 128 + rowof)

    msrc = np.concatenate([row, np.arange(N, dtype=np.int64)])
    mdst = np.concatenate([col, np.arange(N, dtype=np.int64)])
    mscale = np.where(np.arange(msrc.size) < row.size,
                      dis[msrc] * dis[mdst], dis[mdst] ** 2).astype(np.float32)
    mcore = coreof[mdst]
    mslot = slotof[mdst]
    mrow = rowof[mdst].astype(np.float32)

    def make_layer(idxs, half, split_base):
        key = (mcore * NT + mslot) * 2 + half
        perm = np.argsort(key, kind='stable')
        ks, ids, rs, scs = key[perm], idxs[perm], mrow[perm], mscale[perm]
        bounds = np.searchsorted(ks, np.arange(C * NT * 2 + 1))
        cnt = np.diff(bounds).reshape(C, NT, 2)
        nb = np.ceil(cnt / 128).astype(np.int64).max(axis=0)   # [NT, 2]
        NB = int(nb.sum())
        blk_start = np.zeros((NT, 2), dtype=np.int64)
        ccols = []           # per-(group, half): (blockstart, nblocks, half, tiles)
        pos = 0
        for g in range(0, NT, GROUP):
            tiles = list(range(g, min(g + GROUP, NT)))
            for h in (0, 1):
                n_here = 0
                for t in tiles:
                    blk_start[t, h] = pos + n_here
                    n_here += int(nb[t, h])
                ccols.append((pos, n_here, h, tiles))
                pos += n_here
        assert pos == NB

        COLS = NB * 8
        idx_arr = np.zeros((C, P, COLS), dtype=np.int16)
        dc_arr = np.zeros((C, P, NB), dtype=np.float32)
        sc_arr = np.zeros((C, P, NB), dtype=np.float32)
        for k in range(C):
            flat = np.zeros(NB * 128, dtype=np.int64)
            for t in range(NT):
                for h in (0, 1):
                    j = (k * NT + t) * 2 + h
                    lo, hi = int(bounds[j]), int(bounds[j + 1])
                    n = hi - lo
                    b0 = int(blk_start[t, h])
                    flat[b0 * 128:b0 * 128 + n] = ids[lo:hi] - h * split_base
                    mm = np.arange(n)
                    dc_arr[k, mm % 128, b0 + mm // 128] = rs[lo:hi]
                    sc_arr[k, mm % 128, b0 + mm // 128] = scs[lo:hi]
            w = flat.reshape(-1, 16).T.astype(np.int16)
            idx_arr[k] = np.tile(w, (8, 1))
        return dict(nb=nb, NB=NB, COLS=COLS, idx_arr=idx_arr, dc_arr=dc_arr,
                    sc_arr=sc_arr, blk_start=blk_start, ccols=ccols)

    idx1 = msrc
    idx2 = vrow[msrc]
    L1 = make_layer(idx1, (idx1 >= XSPLIT).astype(np.int64), XSPLIT)
    L2 = make_layer(idx2, (idx2 >= VSPLIT).astype(np.int64), VSPLIT)
    return L1, L2, dict(coreof=coreof, slotof=slotof, rowof=rowof)


def kernel(x, edge_index, W1, b1, W2, b2):
    import concourse.bass as bass
    import concourse.bacc as bacc
    import concourse.mybir as mybir
    from concourse import tile
    from concourse import library_config
    from concourse.bass_utils import run_bass_kernel_spmd

    x = np.asarray(x, dtype=np.float32)
    W1 = np.asarray(W1, dtype=np.float32)
    W2 = np.asarray(W2, dtype=np.float32)
    b1 = np.asarray(b1, dtype=np.float32)
    b2 = np.asarray(b2, dtype=np.float32)

    L1, L2, meta = _build(edge_index)

    xtab_np = np.zeros((NFULL, D), dtype=np.float32)
    xtab_np[:N] = x
    iota_np = np.tile(np.arange(P, dtype=np.float32), (P, 1))
    ident_np = np.eye(P, dtype=np.float32)

    NBG1 = max(L1['ccols'][i][1] + L1['ccols'][i + 1][1]
               for i in range(0, len(L1['ccols']), 2))
    NBG2 = max(L2['ccols'][i][1] + L2['ccols'][i + 1][1]
               for i in range(0, len(L2['ccols']), 2))

    # ---- device program ----
    nc = bacc.Bacc(None, target_bir_lowering=False, num_swdge_queues=4)
    dt = mybir.dt
    xtab = nc.declare_dram_parameter("xtab", [NFULL, D], dt.float32, isOutput=False)
    identp = nc.declare_dram_parameter("identp", [P, P], dt.float32, isOutput=False)
    iotap = nc.declare_dram_parameter("iotap", [P, P], dt.float32, isOutput=False)
    idx1p = nc.declare_dram_parameter("idx1p", [P, L1['COLS']], dt.int16, isOutput=False)
    idx2p = nc.declare_dram_parameter("idx2p", [P, L2['COLS']], dt.int16, isOutput=False)
    dc1p = nc.declare_dram_parameter("dc1p", [P, L1['NB']], dt.float32, isOutput=False)
    sc1p = nc.declare_dram_parameter("sc1p", [P, L1['NB']], dt.float32, isOutput=False)
    dc2p = nc.declare_dram_parameter("dc2p", [P, L2['NB']], dt.float32, isOutput=False)
    sc2p = nc.declare_dram_parameter("sc2p", [P, L2['NB']], dt.float32, isOutput=False)
    w1p = nc.declare_dram_parameter("w1p", [D, D], dt.float32, isOutput=False)
    w2p = nc.declare_dram_parameter("w2p", [D, D], dt.float32, isOutput=False)
    b1p = nc.declare_dram_parameter("b1p", [D, 1], dt.float32, isOutput=False)
    b2p = nc.declare_dram_parameter("b2p", [D, 1], dt.float32, isOutput=False)
    out_sh = nc.declare_dram_parameter("out_sh", [NT * 128, D], dt.float32,
                                       isOutput=True)

    v_shp = [nc.dram_tensor(f"v_sh{i}", [PIECE_ROWS[i], D], dt.float32)
             for i in range(4)]
    vfull = nc.dram_tensor("vfull", [NFULL, D], dt.float32, addr_space="Shared")

    rg = [list(range(C))]
    Copy = mybir.ActivationFunctionType.Copy
    Ident = mybir.ActivationFunctionType.Identity

    with tile.TileContext(nc) as tc:
        with tc.tile_pool(name="const", bufs=1) as cp, \
             tc.tile_pool(name="gp", bufs=2) as gpool, \
             tc.tile_pool(name="selp", bufs=4) as selp, \
             tc.tile_pool(name="ep", bufs=4) as ep, \
             tc.tile_pool(name="psA", bufs=2, space="PSUM") as psA, \
             tc.tile_pool(name="psB1", bufs=2, space="PSUM") as psB1, \
             tc.tile_pool(name="psB2", bufs=2, space="PSUM") as psB2, \
             tc.tile_pool(name="psC", bufs=2, space="PSUM") as psC:

            nc.gpsimd.load_library(library_config.mlp)

            ident = cp.tile([P, P], dt.float32)
            nc.sync.dma_start(out=ident[:], in_=identp[:, :])
            iota = cp.tile([P, P], dt.float32)
            nc.sync.dma_start(out=iota[:], in_=iotap[:, :])
            w1t = cp.tile([D, D], dt.float32)
            nc.sync.dma_start(out=w1t[:], in_=w1p[:, :])
            w2t = cp.tile([D, D], dt.float32)
            nc.sync.dma_start(out=w2t[:], in_=w2p[:, :])
            b1t = cp.tile([D, 1], dt.float32)
            nc.sync.dma_start(out=b1t[:], in_=b1p[:, :])
            b2t = cp.tile([D, 1], dt.float32)
            nc.sync.dma_start(out=b2t[:], in_=b2p[:, :])
            idx1t = cp.tile([P, L1['COLS']], dt.int16)
            nc.sync.dma_start(out=idx1t[:], in_=idx1p[:, :])
            idx2t = cp.tile([P, L2['COLS']], dt.int16)
            nc.sync.dma_start(out=idx2t[:], in_=idx2p[:, :])
            dc1t = cp.tile([P, L1['NB']], dt.float32)
            nc.sync.dma_start(out=dc1t[:], in_=dc1p[:, :])
            sc1t = cp.tile([P, L1['NB']], dt.float32)
            nc.sync.dma_start(out=sc1t[:], in_=sc1p[:, :])
            dc2t = cp.tile([P, L2['NB']], dt.float32)
            nc.sync.dma_start(out=dc2t[:], in_=dc2p[:, :])
            sc2t = cp.tile([P, L2['NB']], dt.float32)
            nc.sync.dma_start(out=sc2t[:], in_=sc2p[:, :])

            def layer(L, tabA, tabB, idxt, dct, sct, wt, bt, NBGmax,
                      dest_of, post_tile=None):
                nb, blk_start = L['nb'], L['blk_start']
                # per-(group,half) idx column offsets
                colpos = {}
                cpos = 0
                for (b0, nbl, h, tiles) in L['ccols']:
                    colpos[(tiles[0], h)] = cpos
                    cpos += nbl * 8
                groups = []
                for i in range(0, len(L['ccols']), 2):
                    groups.append((L['ccols'][i], L['ccols'][i + 1]))
                CHUNK = 8            # blocks per gather call (ring limit)
                qctr = [0]

                def gcalls(tab, c0, goff, nblk, gbuf):
                    for off in range(0, nblk, CHUNK):
                        m = min(CHUNK, nblk - off)
                        nc.gpsimd.dma_gather(
                            gbuf[:, goff + off:goff + off + m, :], tab,
                            idxt[:, c0 + off * 8:c0 + (off + m) * 8],
                            m * 128, m * 128, D, queue_num=qctr[0] % 4)
                        qctr[0] += 1

                for (cA, cB) in groups:
                    b0A, nA, _, tiles = cA
                    b0B, nB_, _, _ = cB
                    gbuf = gpool.tile([P, NBGmax, D], dt.float32, tag="g")
                    if nA > 0:
                        gcalls(tabA, colpos[(tiles[0], 0)], 0, nA, gbuf)
                    if nB_ > 0:
                        gcalls(tabB, colpos[(tiles[0], 1)], nA, nB_, gbuf)
                    for t in tiles:
                        blocks = []
                        for h, base in ((0, b0A), (1, b0A)):
                            bs = int(blk_start[t, h])
                            for i in range(int(nb[t, h])):
                                blocks.append(bs + i)
                        acc = psA.tile([P, D], dt.float32)
                        nblk = len(blocks)
                        for j, blk in enumerate(blocks):
                            sel = selp.tile([P, P], dt.float32, tag="s")
                            nc.vector.tensor_scalar(
                                out=sel[:], in0=iota[:],
                                scalar1=dct[:, blk:blk + 1],
                                scalar2=sct[:, blk:blk + 1],
                                op0=mybir.AluOpType.is_equal,
                                op1=mybir.AluOpType.mult)
                            nc.tensor.matmul(acc[:], lhsT=sel[:],
                                             rhs=gbuf[:, blk - b0A, :],
                                             start=(j == 0),
                                             stop=(j == nblk - 1))
                        asb = ep.tile([P, D], dt.float32, tag="a")
                        nc.scalar.activation(out=asb[:], in_=acc[:], func=Copy)
                        tr1 = psB1.tile([D, P], dt.float32)
                        nc.tensor.transpose(tr1[:], asb[:], ident[:])
                        ct = ep.tile([D, P], dt.float32, tag="c")
                        nc.scalar.activation(out=ct[:], in_=tr1[:], func=Copy)
                        pv = psC.tile([D, P], dt.float32)
                        nc.tensor.matmul(pv[:], lhsT=wt[:], rhs=ct[:],
                                         start=True, stop=True)
                        vt = ep.tile([D, P], dt.float32, tag="v")
                        nc.scalar.activation(out=vt[:], in_=pv[:], func=Ident,
                                             bias=bt[:, 0:1])
                        tr2 = psB2.tile([P, D], dt.float32)
                        nc.tensor.matmul(tr2[:], lhsT=vt[:], rhs=ident[:D, :D],
                                         is_transpose=True)
                        vsb = ep.tile([P, D], dt.float32, tag="o")
                        nc.vector.tensor_copy(out=vsb[:], in_=tr2[:])
                        dtens, doff = dest_of(t)
                        nc.sync.dma_start(out=dtens[doff:doff + P, :], in_=vsb[:])
                        if post_tile is not None:
                            post_tile(t)

            def v_dest(t):
                p = 0
                while t >= TB[p + 1]:
                    p += 1
                return v_shp[p], (t - TB[p]) * P

            def fire_ag(t):
                for i in range(4):
                    if t == TB[i + 1] - 1:
                        lo = int(REGION[i])
                        hi = int(REGION[i + 1])
                        nc.gpsimd.collective_compute(
                            "AllGather", mybir.AluOpType.bypass,
                            replica_groups=rg,
                            ins=[v_shp[i][:]], outs=[vfull[lo:hi, :]])

            layer(L1, xtab[0:XSPLIT, :], xtab[XSPLIT:NFULL, :], idx1t,
                  dc1t, sc1t, w1t, b1t, NBG1, v_dest, fire_ag)
            layer(L2, vfull[0:VSPLIT, :], vfull[VSPLIT:NFULL, :], idx2t,
                  dc2t, sc2t, w2t, b2t, NBG2,
                  lambda t: (out_sh, t * P))

    nc.compile()

    in_maps = []
    for k in range(C):
        in_maps.append({
            "xtab": xtab_np, "identp": ident_np, "iotap": iota_np,
            "idx1p": L1['idx_arr'][k], "idx2p": L2['idx_arr'][k],
            "dc1p": L1['dc_arr'][k], "sc1p": L1['sc_arr'][k],
            "dc2p": L2['dc_arr'][k], "sc2p": L2['sc_arr'][k],
            "w1p": W1, "w2p": W2,
            "b1p": b1.reshape(D, 1), "b2p": b2.reshape(D, 1),
        })
    global _compiled
    _compiled = (nc, in_maps)
    res = run_bass_kernel_spmd(nc, in_maps, list(range(C)))
    out = np.empty((N, D), dtype=np.float32)
    co, sl, ro = meta['coreof'], meta['slotof'], meta['rowof']
    allout = np.stack([res.results[k]["out_sh"] for k in range(C)])
    out = allout[co, sl * 128 + ro]
    return out


def profile_last():
    """Re-run the last compiled program with NTFF tracing; returns exec ns."""
    from concourse.bass_utils import run_bass_kernel_spmd
    assert _compiled is not None
    nc, in_maps = _compiled
    r = run_bass_kernel_spmd(nc, in_maps, list(range(C)), trace=True)
    return r.exec_time_ns


# revision 23
# speedup vs baseline: 2.2503x; 1.3792x over previous
"""Two-layer GCN block (PyG GCNConv x2) on 8 trn2 NeuronCores.

Per layer: out = D^-1/2 (A+I) D^-1/2 (h W) + b. The symmetric norm is
factorized: dis_s folds into the gather table (xu = dis*x for layer 1;
layer 1 writes v = dis*out1 for layer 2), dis_d is a per-tile lane scale
applied on the PSUM->SBUF copy. Dests are binned into 392 tiles of <=128
(by half-A source count then degree, so per-tile slot counts are tight),
dealt to 8 cores x 49 slots with one common schedule. Messages sit in a
slot grid [lane=dest, slot]: bulk-gathered in bf16 with gpsimd.dma_gather
(<=1024 idx per call, round-robin over 4 SWDGE queues), accumulated in
PSUM by identity matmuls, then (dis_d*acc)@W + b per tile. Both layers
share one table layout (assignment-ordered, piece-major) so one int16
index array serves both; layer-1 tiles are AllGathered in 4 pieces.
"""
import numpy as np

import sys
sys.path.insert(0, '/root/.axon_site')
sys.path.insert(0, '/opt/trn_rl_repo')

N = 50000
D = 64
DP = 128                     # padded feature row (bf16, 256B elems)
C = 8
NT = 49
NBINS = C * NT               # 392
NFULL = NBINS * 128          # 50176
P = 128
GROUP = 7
TB = [0, 13, 25, 37, 49]     # AllGather piece boundaries (slots)
VSPLIT_PIECE = 2

PIECE_ROWS = [(TB[i + 1] - TB[i]) * 128 for i in range(4)]
REGION = np.concatenate([[0], np.cumsum([C * r for r in PIECE_ROWS])])
VSPLIT = int(REGION[VSPLIT_PIECE])

CHUNK = 8                    # blocks per dma_gather call (ring limit)

_compiled = None


SPLIT_SLOT = TB[VSPLIT_PIECE]          # 25


def _fill(order, slots, coreof, slotof, laneof):
    """Fill the (core, slot in `slots`, lane) positions with dests in
    `order`. Lane 127 of the first slot's tiles and of the last slot's
    tiles is reserved ghost (zero table row); shortfall lands at the end
    as extra ghosts."""
    pos = 0
    first, last = slots[0], slots[-1]
    for slot in slots:
        cap = 127 if slot in (first, last) else 128
        take = min(cap * C, len(order) - pos)
        if take <= 0:
            continue
        ds = order[pos:pos + take]
        i = np.arange(take)
        coreof[ds] = i % C
        slotof[ds] = slot
        laneof[ds] = i // C
        pos += take
    assert pos == len(order), (pos, len(order))


def _vrow(coreof, slotof, laneof):
    pieceof = np.digitize(slotof, TB[1:])
    pr = np.array(PIECE_ROWS, dtype=np.int64)
    tbl = np.array(TB[:4], dtype=np.int64)
    return (REGION[pieceof] + coreof * pr[pieceof]
            + (slotof - tbl[pieceof]) * 128 + laneof)


def _build(edge_index):
    row = np.asarray(edge_index[0], dtype=np.int64)
    col = np.asarray(edge_index[1], dtype=np.int64)

    deg = np.bincount(col, minlength=N).astype(np.float32) + 1.0
    dis = (1.0 / np.sqrt(deg)).astype(np.float32)

    # msgs: edges + self loops (dest d receives source s; scale folded away)
    msrc = np.concatenate([row, np.arange(N, dtype=np.int64)])
    mdst = np.concatenate([col, np.arange(N, dtype=np.int64)])

    # phase 1: half-A membership = top-degree nodes (fixed from here on)
    order = np.argsort(-deg, kind='stable')
    acap = C * (127 * 2 + 128 * (SPLIT_SLOT - 2))
    Aset = order[:acap]
    Bset = order[acap:]
    inA = np.zeros(N, dtype=bool)
    inA[Aset] = True
    # exact half-A source count per dest (self loop included)
    lA = np.bincount(mdst, weights=inA[msrc].astype(np.float64),
                     minlength=N).astype(np.int64)
    # phase 2: within each half, order by (lA, degree) for tight slot maxes
    key = lA * 1000 + deg.astype(np.int64)
    co = np.empty(N, dtype=np.int64)
    sl = np.empty(N, dtype=np.int64)
    la = np.empty(N, dtype=np.int64)
    _fill(Aset[np.argsort(-key[Aset], kind='stable')],
          list(range(0, SPLIT_SLOT)), co, sl, la)
    _fill(Bset[np.argsort(-key[Bset], kind='stable')],
          list(range(SPLIT_SLOT, NT)), co, sl, la)
    vr = _vrow(co, sl, la)

    # final per-msg placement
    half = (vr[msrc] >= VSPLIT).astype(np.int64)
    idxs = vr[msrc] - half * VSPLIT

    # per (core, slot, lane, half) counts -> slot positions
    lane_key = ((co[mdst] * NT + sl[mdst]) * 128 + la[mdst]) * 2 + half
    perm = np.argsort(lane_key, kind='stable')
    ks, ids = lane_key[perm], idxs[perm]
    bounds = np.searchsorted(ks, np.arange(C * NT * 128 * 2 + 1))
    cnt = np.diff(bounds).reshape(C, NT, 128, 2)
    nb = cnt.max(axis=(0, 2))                     # [NT, 2] common schedule
    NB = int(nb.sum())

    blk_start = np.zeros((NT, 2), dtype=np.int64)
    ccols = []
    pos = 0
    for g in range(0, NT, GROUP):
        tiles = list(range(g, min(g + GROUP, NT)))
        for h in (0, 1):
            n_here = 0
            for t in tiles:
                blk_start[t, h] = pos + n_here
                n_here += int(nb[t, h])
            ccols.append((pos, n_here, h, tiles))
            pos += n_here
    assert pos == NB

    # dummy (padding) indices: guaranteed zero rows per half, per core
    dumA = _vrow(np.arange(C), np.zeros(C, dtype=np.int64),
                 np.full(C, 127, dtype=np.int64))
    dumB = _vrow(np.arange(C), np.full(C, 48, dtype=np.int64),
                 np.full(C, 127, dtype=np.int64)) - VSPLIT
    assert dumA.max() < VSPLIT and dumB.min() >= 0

    COLS = NB * 8
    idx_arr = np.zeros((C, P, COLS), dtype=np.int16)
    disd_arr = np.zeros((C, P, NT), dtype=np.float32)
    disd_arr[co, la, sl] = dis

    # vectorized slot placement: msg rank within its (core,slot,lane,half)
    swi = np.arange(ks.size) - bounds[ks]
    mk = ks >> 1
    mh = ks & 1
    mp = mk % 128
    mt = (mk // 128) % NT
    mcore = mk // (128 * NT)
    flat_pos = (blk_start[mt, mh] + swi) * 128 + mp
    # half of each block position (for dummy fill)
    halfof = np.zeros(NB, dtype=np.int64)
    for t in range(NT):
        halfof[blk_start[t, 1]:blk_start[t, 1] + int(nb[t, 1])] = 1
    hrep = np.repeat(halfof, 128)
    for k in range(C):
        flat = np.empty(NB * 128, dtype=np.int64)
        flat[:] = -1
        mkk = mcore == k
        flat[flat_pos[mkk]] = ids[mkk]
        mask = flat < 0
        flat[mask & (hrep == 0)] = dumA[k]
        flat[mask & (hrep == 1)] = dumB[k]
        w = flat.reshape(-1, 16).T.astype(np.int16)
        idx_arr[k] = np.tile(w, (8, 1))
    return dict(nb=nb, NB=NB, COLS=COLS, idx_arr=idx_arr, disd_arr=disd_arr,
                blk_start=blk_start, ccols=ccols,
                co=co, sl=sl, la=la, vr=vr, dis=dis)


def kernel(x, edge_index, W1, b1, W2, b2):
    import concourse.bass as bass  # noqa: F401
    import concourse.bacc as bacc
    import concourse.mybir as mybir
    from concourse import tile
    from concourse import library_config
    from concourse.bass_utils import run_bass_kernel_spmd
    import ml_dtypes

    x = np.asarray(x, dtype=np.float32)
    W1 = np.asarray(W1, dtype=np.float32)
    W2 = np.asarray(W2, dtype=np.float32)
    b1 = np.asarray(b1, dtype=np.float32)
    b2 = np.asarray(b2, dtype=np.float32)

    S = _build(edge_index)
    co, sl, la, vr, dis = S['co'], S['sl'], S['la'], S['vr'], S['dis']
    NB, COLS, nb, blk_start, ccols = (S['NB'], S['COLS'], S['nb'],
                                      S['blk_start'], S['ccols'])

    # xu table in assignment order, bf16, padded to 128 features
    xu = np.zeros((NFULL, DP), dtype=ml_dtypes.bfloat16)
    xu[vr, :D] = (dis[:, None] * x).astype(ml_dtypes.bfloat16)

    ident_np = np.eye(P, dtype=np.float32)
    identb_np = np.eye(P, dtype=ml_dtypes.bfloat16)

    NBG = max(ccols[i][1] + ccols[i + 1][1] for i in range(0, len(ccols), 2))

    nc = bacc.Bacc(None, target_bir_lowering=False, num_swdge_queues=4)
    dt = mybir.dt
    xup = nc.declare_dram_parameter("xup", [NFULL, DP], dt.bfloat16, isOutput=False)
    identp = nc.declare_dram_parameter("identp", [P, P], dt.float32, isOutput=False)
    identbp = nc.declare_dram_parameter("identbp", [P, P], dt.bfloat16, isOutput=False)
    idxp = nc.declare_dram_parameter("idxp", [P, COLS], dt.int16, isOutput=False)
    disdp = nc.declare_dram_parameter("disdp", [P, NT], dt.float32, isOutput=False)
    w1p = nc.declare_dram_parameter("w1p", [D, D], dt.bfloat16, isOutput=False)
    w2p = nc.declare_dram_parameter("w2p", [D, D], dt.bfloat16, isOutput=False)
    b1p = nc.declare_dram_parameter("b1p", [D, 1], dt.float32, isOutput=False)
    b2p = nc.declare_dram_parameter("b2p", [D, 1], dt.float32, isOutput=False)
    out_sh = nc.declare_dram_parameter("out_sh", [NT * 128, D], dt.float32,
                                       isOutput=True)
    import os
    DBG = os.environ.get("KDBG") == "1"
    DBG2 = os.environ.get("KDBG") == "2"
    if DBG:
        dbg_v = nc.declare_dram_parameter("dbg_v", [NT * 128, D], dt.bfloat16,
                                          isOutput=True)
    if DBG2:
        dbg_vf = nc.declare_dram_parameter("dbg_vf", [NFULL, DP], dt.bfloat16,
                                           isOutput=True)

    v_shp = [nc.dram_tensor(f"v_sh{i}", [PIECE_ROWS[i], DP], dt.bfloat16)
             for i in range(4)]
    vfull = nc.dram_tensor("vfull", [NFULL, DP], dt.bfloat16,
                           addr_space="Shared")

    rg = [list(range(C))]
    Copy = mybir.ActivationFunctionType.Copy
    Ident = mybir.ActivationFunctionType.Identity

    with tile.TileContext(nc) as tc:
        with tc.tile_pool(name="const", bufs=1) as cp, \
             tc.tile_pool(name="gp", bufs=2) as gpool, \
             tc.tile_pool(name="ep", bufs=4) as ep, \
             tc.tile_pool(name="psA", bufs=2, space="PSUM") as psA, \
             tc.tile_pool(name="psB1", bufs=2, space="PSUM") as psB1, \
             tc.tile_pool(name="psB2", bufs=2, space="PSUM") as psB2, \
             tc.tile_pool(name="psC", bufs=2, space="PSUM") as psC:

            nc.gpsimd.load_library(library_config.mlp)

            ident = cp.tile([P, P], dt.float32)
            nc.sync.dma_start(out=ident[:], in_=identp[:, :])
            identb = cp.tile([P, P], dt.bfloat16)
            nc.sync.dma_start(out=identb[:], in_=identbp[:, :])
            w1t = cp.tile([D, D], dt.bfloat16)
            nc.sync.dma_start(out=w1t[:], in_=w1p[:, :])
            w2t = cp.tile([D, D], dt.bfloat16)
            nc.sync.dma_start(out=w2t[:], in_=w2p[:, :])
            b1t = cp.tile([D, 1], dt.float32)
            nc.sync.dma_start(out=b1t[:], in_=b1p[:, :])
            b2t = cp.tile([D, 1], dt.float32)
            nc.sync.dma_start(out=b2t[:], in_=b2p[:, :])
            idxt = cp.tile([P, COLS], dt.int16)
            h0 = COLS // 2
            nc.sync.dma_start(out=idxt[:, :h0], in_=idxp[:, :h0])
            nc.sync.dma_start(out=idxt[:, h0:], in_=idxp[:, h0:])
            disdt = cp.tile([P, NT], dt.float32)
            nc.sync.dma_start(out=disdt[:], in_=disdp[:, :])

            qctr = [0]

            def layer(tab, wt, bt, scale_out, dest_of, post_tile=None):
                colpos = {}
                cpos = 0
                for (b0, nbl, h, tiles) in ccols:
                    colpos[(tiles[0], h)] = cpos
                    cpos += nbl * 8

                def gcalls(base_lo, c0, goff, nblk, gbuf):
                    for off in range(0, nblk, CHUNK):
                        m = min(CHUNK, nblk - off)
                        nc.gpsimd.dma_gather(
                            gbuf[:, goff + off:goff + off + m, :],
                            tab[base_lo[0]:base_lo[1], :],
                            idxt[:, c0 + off * 8:c0 + (off + m) * 8],
                            m * 128, m * 128, DP, queue_num=qctr[0] % 4)
                        qctr[0] += 1

                groups = [(ccols[i], ccols[i + 1])
                          for i in range(0, len(ccols), 2)]
                for (cA, cB) in groups:
                    b0A, nA, _, tiles = cA
                    b0B, nB_, _, _ = cB
                    gbuf = gpool.tile([P, NBG, DP], dt.bfloat16, tag="g")
                    if nA > 0:
                        gcalls((0, VSPLIT), colpos[(tiles[0], 0)], 0, nA, gbuf)
                    if nB_ > 0:
                        gcalls((VSPLIT, NFULL), colpos[(tiles[0], 1)],
                               nA, nB_, gbuf)
                    for t in tiles:
                        nblk = int(nb[t, 0] + nb[t, 1])
                        acc = psA.tile([P, D], dt.float32)
                        j = 0
                        for h in (0, 1):
                            bs = int(blk_start[t, h]) - b0A
                            for i in range(int(nb[t, h])):
                                nc.tensor.matmul(acc[:], lhsT=identb[:],
                                                 rhs=gbuf[:, bs + i, 0:D],
                                                 start=(j == 0),
                                                 stop=(j == nblk - 1))
                                j += 1
                        # asb = dis_d * acc  (bf16)
                        asb = ep.tile([P, D], dt.bfloat16, tag="a")
                        nc.scalar.activation(out=asb[:], in_=acc[:], func=Copy,
                                             scale=disdt[:, t:t + 1])
                        tr1 = psB1.tile([D, P], dt.bfloat16)
                        nc.tensor.transpose(tr1[:], asb[:], identb[:])
                        ct = ep.tile([D, P], dt.bfloat16, tag="c")
                        nc.scalar.activation(out=ct[:], in_=tr1[:], func=Copy)
                        pv = psC.tile([D, P], dt.float32)
                        nc.tensor.matmul(pv[:], lhsT=wt[:], rhs=ct[:],
                                         start=True, stop=True)
                        vt = ep.tile([D, P], dt.bfloat16, tag="v")
                        nc.scalar.activation(out=vt[:], in_=pv[:], func=Ident,
                                             bias=bt[:, 0:1])
                        tr2 = psB2.tile([P, D], dt.bfloat16)
                        nc.tensor.matmul(tr2[:], lhsT=vt[:],
                                         rhs=identb[:D, :D],
                                         is_transpose=True)
                        dest_of(t, tr2)
                        if post_tile is not None:
                            post_tile(t)

            def v_dest(t, tr2):
                p = 0
                while t >= TB[p + 1]:
                    p += 1
                off = (t - TB[p]) * P
                vsb = ep.tile([P, D], dt.bfloat16, tag="o")
                # v = dis_d * out1   (ghost lanes -> 0)
                nc.scalar.activation(out=vsb[:], in_=tr2[:], func=Copy,
                                     scale=disdt[:, t:t + 1])
                nc.sync.dma_start(out=v_shp[p][off:off + P, 0:D], in_=vsb[:])
                if DBG:
                    nc.sync.dma_start(out=dbg_v[t * P:(t + 1) * P, :],
                                      in_=vsb[:])

            def o_dest(t, tr2):
                vsb = ep.tile([P, D], dt.float32, tag="o2")
                nc.scalar.activation(out=vsb[:], in_=tr2[:], func=Copy)
                nc.sync.dma_start(out=out_sh[t * P:(t + 1) * P, :], in_=vsb[:])

            def fire_ag(t):
                for i in range(4):
                    if t == TB[i + 1] - 1:
                        lo, hi = int(REGION[i]), int(REGION[i + 1])
                        nc.gpsimd.collective_compute(
                            "AllGather", mybir.AluOpType.bypass,
                            replica_groups=rg,
                            ins=[v_shp[i][:]], outs=[vfull[lo:hi, :]])

            layer(xup, w1t, b1t, True, v_dest, fire_ag)
            tc.strict_bb_all_engine_barrier()
            if DBG2:
                for i in range(0, NFULL, 1792):
                    hi = min(i + 1792, NFULL)
                    st = ep.tile([P, (1792 // P) * DP], dt.bfloat16, tag="dbg")
                    nc.sync.dma_start(out=st[:, :(hi - i) // P * DP],
                                      in_=vfull[i:hi, :])
                    nc.sync.dma_start(out=dbg_vf[i:hi, :],
                                      in_=st[:, :(hi - i) // P * DP])
            layer(vfull, w2t, b2t, False, o_dest)

    nc.compile()

    in_maps = []
    for k in range(C):
        in_maps.append({
            "xup": xu, "identp": ident_np,
            "identbp": identb_np,
            "idxp": S['idx_arr'][k], "disdp": S['disd_arr'][k],
            "w1p": W1.astype(ml_dtypes.bfloat16),
            "w2p": W2.astype(ml_dtypes.bfloat16),
            "b1p": b1.reshape(D, 1), "b2p": b2.reshape(D, 1),
        })
    global _compiled
    _compiled = (nc, in_maps)
    res = run_bass_kernel_spmd(nc, in_maps, list(range(C)))
    allout = np.stack([res.results[k]["out_sh"] for k in range(C)])
    out = allout[co, sl * 128 + la]
    return np.ascontiguousarray(out)


def profile_last():
    """Re-run the last compiled program with NTFF tracing; returns exec ns."""
    from concourse.bass_utils import run_bass_kernel_spmd
    assert _compiled is not None
    nc, in_maps = _compiled
    r = run_bass_kernel_spmd(nc, in_maps, list(range(C)), trace=True)
    return r.exec_time_ns


# revision 29
# speedup vs baseline: 2.3021x; 1.0230x over previous
"""Two-layer GCN block (PyG GCNConv x2) on 8 trn2 NeuronCores.

Per layer: out = D^-1/2 (A+I) D^-1/2 (h W) + b. The symmetric norm is
factorized: dis_s folds into the gather table (xu = dis*x for layer 1;
layer 1 writes v = dis*out1 for layer 2), dis_d is a per-tile lane scale
applied on the PSUM->SBUF copy. Dests are binned into 392 tiles of <=128
(by half-A source count then degree, so per-tile slot counts are tight),
dealt to 8 cores x 49 slots with one common schedule. Messages sit in a
slot grid [lane=dest, slot]: bulk-gathered in bf16 with gpsimd.dma_gather
(<=1024 idx per call, round-robin over 4 SWDGE queues), accumulated in
PSUM by identity matmuls, then (dis_d*acc)@W + b per tile. Both layers
share one table layout (assignment-ordered, piece-major) so one int16
index array serves both; layer-1 tiles are AllGathered in 4 pieces.
"""
import numpy as np

import sys
sys.path.insert(0, '/root/.axon_site')
sys.path.insert(0, '/opt/trn_rl_repo')

N = 50000
D = 64
DP = 128                     # padded feature row (bf16, 256B elems)
C = 8
NT = 49
NBINS = C * NT               # 392
NFULL = NBINS * 128          # 50176
P = 128
GROUP = 7
TB = [0, 13, 25, 37, 49]     # AllGather piece boundaries (slots)
VSPLIT_PIECE = 2

PIECE_ROWS = [(TB[i + 1] - TB[i]) * 128 for i in range(4)]
REGION = np.concatenate([[0], np.cumsum([C * r for r in PIECE_ROWS])])
VSPLIT = int(REGION[VSPLIT_PIECE])

CHUNK = 8                    # blocks per dma_gather call (ring limit)

_compiled = None


SPLIT_SLOT = TB[VSPLIT_PIECE]          # 25


def _fill(order, slots, coreof, slotof, laneof):
    """Fill the (core, slot in `slots`, lane) positions with dests in
    `order`. Lane 127 of the first slot's tiles and of the last slot's
    tiles is reserved ghost (zero table row); shortfall lands at the end
    as extra ghosts."""
    pos = 0
    first, last = slots[0], slots[-1]
    for slot in slots:
        cap = 127 if slot in (first, last) else 128
        take = min(cap * C, len(order) - pos)
        if take <= 0:
            continue
        ds = order[pos:pos + take]
        i = np.arange(take)
        coreof[ds] = i % C
        slotof[ds] = slot
        laneof[ds] = i // C
        pos += take
    assert pos == len(order), (pos, len(order))


def _vrow(coreof, slotof, laneof):
    pieceof = np.digitize(slotof, TB[1:])
    pr = np.array(PIECE_ROWS, dtype=np.int64)
    tbl = np.array(TB[:4], dtype=np.int64)
    return (REGION[pieceof] + coreof * pr[pieceof]
            + (slotof - tbl[pieceof]) * 128 + laneof)


def _build(edge_index):
    row = np.asarray(edge_index[0], dtype=np.int64)
    col = np.asarray(edge_index[1], dtype=np.int64)

    deg = np.bincount(col, minlength=N).astype(np.float32) + 1.0
    dis = (1.0 / np.sqrt(deg)).astype(np.float32)

    # msgs: edges + self loops (dest d receives source s; scale folded away)
    msrc = np.concatenate([row, np.arange(N, dtype=np.int64)])
    mdst = np.concatenate([col, np.arange(N, dtype=np.int64)])

    # phase 1: half-A membership = top-degree nodes (fixed from here on)
    order = np.argsort(-deg, kind='stable')
    acap = C * (127 * 2 + 128 * (SPLIT_SLOT - 2))
    Aset = order[:acap]
    Bset = order[acap:]
    inA = np.zeros(N, dtype=bool)
    inA[Aset] = True
    # exact half-A source count per dest (self loop included)
    lA = np.bincount(mdst, weights=inA[msrc].astype(np.float64),
                     minlength=N).astype(np.int64)
    # phase 2: within each half, order by (lA, degree) for tight slot maxes
    key = lA * 1000 + deg.astype(np.int64)
    co = np.empty(N, dtype=np.int64)
    sl = np.empty(N, dtype=np.int64)
    la = np.empty(N, dtype=np.int64)
    _fill(Aset[np.argsort(-key[Aset], kind='stable')],
          list(range(0, SPLIT_SLOT)), co, sl, la)
    _fill(Bset[np.argsort(-key[Bset], kind='stable')],
          list(range(SPLIT_SLOT, NT)), co, sl, la)
    vr = _vrow(co, sl, la)

    # final per-msg placement
    half = (vr[msrc] >= VSPLIT).astype(np.int64)
    idxs = vr[msrc] - half * VSPLIT

    # per (core, slot, lane, half) counts -> slot positions
    lane_key = ((co[mdst] * NT + sl[mdst]) * 128 + la[mdst]) * 2 + half
    perm = np.argsort(lane_key, kind='stable')
    ks, ids = lane_key[perm], idxs[perm]
    bounds = np.searchsorted(ks, np.arange(C * NT * 128 * 2 + 1))
    cnt = np.diff(bounds).reshape(C, NT, 128, 2)
    nb = cnt.max(axis=(0, 2))                     # [NT, 2] common schedule
    NB = int(nb.sum())

    blk_start = np.zeros((NT, 2), dtype=np.int64)
    ccols = []
    pos = 0
    for g in range(0, NT, GROUP):
        tiles = list(range(g, min(g + GROUP, NT)))
        for h in (0, 1):
            n_here = 0
            for t in tiles:
                blk_start[t, h] = pos + n_here
                n_here += int(nb[t, h])
            ccols.append((pos, n_here, h, tiles))
            pos += n_here
    assert pos == NB

    # dummy (padding) indices: guaranteed zero rows per half, per core
    dumA = _vrow(np.arange(C), np.zeros(C, dtype=np.int64),
                 np.full(C, 127, dtype=np.int64))
    dumB = _vrow(np.arange(C), np.full(C, 48, dtype=np.int64),
                 np.full(C, 127, dtype=np.int64)) - VSPLIT
    assert dumA.max() < VSPLIT and dumB.min() >= 0

    COLS = NB * 8
    idx_arr = np.zeros((C, P, COLS), dtype=np.int16)
    disd_arr = np.zeros((C, P, NT), dtype=np.float32)
    disd_arr[co, la, sl] = dis

    # vectorized slot placement: msg rank within its (core,slot,lane,half)
    swi = np.arange(ks.size) - bounds[ks]
    mk = ks >> 1
    mh = ks & 1
    mp = mk % 128
    mt = (mk // 128) % NT
    mcore = mk // (128 * NT)
    flat_pos = (blk_start[mt, mh] + swi) * 128 + mp
    # half of each block position (for dummy fill)
    halfof = np.zeros(NB, dtype=np.int64)
    for t in range(NT):
        halfof[blk_start[t, 1]:blk_start[t, 1] + int(nb[t, 1])] = 1
    hrep = np.repeat(halfof, 128)
    for k in range(C):
        flat = np.empty(NB * 128, dtype=np.int64)
        flat[:] = -1
        mkk = mcore == k
        flat[flat_pos[mkk]] = ids[mkk]
        mask = flat < 0
        flat[mask & (hrep == 0)] = dumA[k]
        flat[mask & (hrep == 1)] = dumB[k]
        w = flat.reshape(-1, 16).T.astype(np.int16)
        idx_arr[k] = np.tile(w, (8, 1))
    return dict(nb=nb, NB=NB, COLS=COLS, idx_arr=idx_arr, disd_arr=disd_arr,
                blk_start=blk_start, ccols=ccols,
                co=co, sl=sl, la=la, vr=vr, dis=dis)


def kernel(x, edge_index, W1, b1, W2, b2):
    import concourse.bass as bass  # noqa: F401
    import concourse.bacc as bacc
    import concourse.mybir as mybir
    from concourse import tile
    from concourse import library_config
    from concourse.bass_utils import run_bass_kernel_spmd
    import ml_dtypes

    x = np.asarray(x, dtype=np.float32)
    W1 = np.asarray(W1, dtype=np.float32)
    W2 = np.asarray(W2, dtype=np.float32)
    b1 = np.asarray(b1, dtype=np.float32)
    b2 = np.asarray(b2, dtype=np.float32)

    S = _build(edge_index)
    co, sl, la, vr, dis = S['co'], S['sl'], S['la'], S['vr'], S['dis']
    NB, COLS, nb, blk_start, ccols = (S['NB'], S['COLS'], S['nb'],
                                      S['blk_start'], S['ccols'])

    # xu table in assignment order, bf16, padded to 128 features
    xu = np.zeros((NFULL, DP), dtype=ml_dtypes.bfloat16)
    xu[vr, :D] = (dis[:, None] * x).astype(ml_dtypes.bfloat16)

    ident_np = np.eye(P, dtype=np.float32)
    identb_np = np.eye(P, dtype=ml_dtypes.bfloat16)

    NBG = max(ccols[i][1] + ccols[i + 1][1] for i in range(0, len(ccols), 2))

    nc = bacc.Bacc(None, target_bir_lowering=False, num_swdge_queues=4)
    dt = mybir.dt
    xup = nc.declare_dram_parameter("xup", [NFULL, DP], dt.bfloat16, isOutput=False)
    identp = nc.declare_dram_parameter("identp", [P, P], dt.float32, isOutput=False)
    identbp = nc.declare_dram_parameter("identbp", [P, P], dt.bfloat16, isOutput=False)
    idxp = nc.declare_dram_parameter("idxp", [P, COLS], dt.int16, isOutput=False)
    disdp = nc.declare_dram_parameter("disdp", [P, NT], dt.float32, isOutput=False)
    w1p = nc.declare_dram_parameter("w1p", [D, D], dt.bfloat16, isOutput=False)
    w2p = nc.declare_dram_parameter("w2p", [D, D], dt.bfloat16, isOutput=False)
    b1p = nc.declare_dram_parameter("b1p", [D, 1], dt.float32, isOutput=False)
    b2p = nc.declare_dram_parameter("b2p", [D, 1], dt.float32, isOutput=False)
    out_sh = nc.declare_dram_parameter("out_sh", [NT * 128, D], dt.float32,
                                       isOutput=True)
    import os
    DBG = os.environ.get("KDBG") == "1"
    DBG2 = os.environ.get("KDBG") == "2"
    if DBG:
        dbg_v = nc.declare_dram_parameter("dbg_v", [NT * 128, D], dt.bfloat16,
                                          isOutput=True)
    if DBG2:
        dbg_vf = nc.declare_dram_parameter("dbg_vf", [NFULL, DP], dt.bfloat16,
                                           isOutput=True)

    v_shp = [nc.dram_tensor(f"v_sh{i}", [PIECE_ROWS[i], DP], dt.bfloat16)
             for i in range(4)]
    vfull = nc.dram_tensor("vfull", [NFULL, DP], dt.bfloat16,
                           addr_space="Shared")

    rg = [list(range(C))]
    Copy = mybir.ActivationFunctionType.Copy
    Ident = mybir.ActivationFunctionType.Identity

    with tile.TileContext(nc) as tc:
        with tc.tile_pool(name="const", bufs=1) as cp, \
             tc.tile_pool(name="gp", bufs=3) as gpool, \
             tc.tile_pool(name="ep", bufs=4) as ep, \
             tc.tile_pool(name="psA", bufs=2, space="PSUM") as psA, \
             tc.tile_pool(name="psB1", bufs=2, space="PSUM") as psB1, \
             tc.tile_pool(name="psB2", bufs=2, space="PSUM") as psB2, \
             tc.tile_pool(name="psC", bufs=2, space="PSUM") as psC:

            nc.gpsimd.load_library(library_config.mlp)

            ident = cp.tile([P, P], dt.float32)
            nc.sync.dma_start(out=ident[:], in_=identp[:, :])
            identb = cp.tile([P, P], dt.bfloat16)
            nc.sync.dma_start(out=identb[:], in_=identbp[:, :])
            w1t = cp.tile([D, D], dt.bfloat16)
            nc.sync.dma_start(out=w1t[:], in_=w1p[:, :])
            w2t = cp.tile([D, D], dt.bfloat16)
            nc.sync.dma_start(out=w2t[:], in_=w2p[:, :])
            b1t = cp.tile([D, 1], dt.float32)
            nc.sync.dma_start(out=b1t[:], in_=b1p[:, :])
            b2t = cp.tile([D, 1], dt.float32)
            nc.sync.dma_start(out=b2t[:], in_=b2p[:, :])
            idxt = cp.tile([P, COLS], dt.int16)
            h0 = COLS // 2
            nc.sync.dma_start(out=idxt[:, :h0], in_=idxp[:, :h0])
            nc.sync.dma_start(out=idxt[:, h0:], in_=idxp[:, h0:])
            disdt = cp.tile([P, NT], dt.float32)
            nc.sync.dma_start(out=disdt[:], in_=disdp[:, :])

            qctr = [0]
            ag_insts = [None] * 4

            def layer(tab, wt, bt, scale_out, dest_of, post_tile=None,
                      gather_deps=None):
                colpos = {}
                cpos = 0
                for (b0, nbl, h, tiles) in ccols:
                    colpos[(tiles[0], h)] = cpos
                    cpos += nbl * 8

                def gcalls(base_lo, c0, goff, nblk, gbuf, h):
                    from concourse.tile_rust import add_dep_helper
                    for off in range(0, nblk, CHUNK):
                        m = min(CHUNK, nblk - off)
                        g = nc.gpsimd.dma_gather(
                            gbuf[:, goff + off:goff + off + m, :],
                            tab[base_lo[0]:base_lo[1], :],
                            idxt[:, c0 + off * 8:c0 + (off + m) * 8],
                            m * 128, m * 128, DP, queue_num=qctr[0] % 4)
                        qctr[0] += 1
                        if gather_deps is not None:
                            for cc in gather_deps(h):
                                add_dep_helper(
                                    g.ins, cc.ins,
                                    reason="L2 gather waits on AllGather")

                groups = [(ccols[i], ccols[i + 1])
                          for i in range(0, len(ccols), 2)]
                for (cA, cB) in groups:
                    b0A, nA, _, tiles = cA
                    b0B, nB_, _, _ = cB
                    gbuf = gpool.tile([P, NBG, DP], dt.bfloat16, tag="g")
                    if nA > 0:
                        gcalls((0, VSPLIT), colpos[(tiles[0], 0)], 0, nA,
                               gbuf, 0)
                    if nB_ > 0:
                        gcalls((VSPLIT, NFULL), colpos[(tiles[0], 1)],
                               nA, nB_, gbuf, 1)
                    for t in tiles:
                        nblk = int(nb[t, 0] + nb[t, 1])
                        acc = psA.tile([P, D], dt.float32)
                        j = 0
                        for h in (0, 1):
                            bs = int(blk_start[t, h]) - b0A
                            for i in range(int(nb[t, h])):
                                nc.tensor.matmul(acc[:], lhsT=identb[:],
                                                 rhs=gbuf[:, bs + i, 0:D],
                                                 start=(j == 0),
                                                 stop=(j == nblk - 1))
                                j += 1
                        # asb = dis_d * acc  (bf16)
                        asb = ep.tile([P, D], dt.bfloat16, tag="a")
                        nc.scalar.activation(out=asb[:], in_=acc[:], func=Copy,
                                             scale=disdt[:, t:t + 1])
                        tr1 = psB1.tile([D, P], dt.bfloat16)
                        nc.tensor.transpose(tr1[:], asb[:], identb[:])
                        ct = ep.tile([D, P], dt.bfloat16, tag="c")
                        nc.scalar.activation(out=ct[:], in_=tr1[:], func=Copy)
                        pv = psC.tile([D, P], dt.float32)
                        nc.tensor.matmul(pv[:], lhsT=wt[:], rhs=ct[:],
                                         start=True, stop=True)
                        vt = ep.tile([D, P], dt.bfloat16, tag="v")
                        nc.scalar.activation(out=vt[:], in_=pv[:], func=Ident,
                                             bias=bt[:, 0:1])
                        tr2 = psB2.tile([P, D], dt.bfloat16)
                        nc.tensor.matmul(tr2[:], lhsT=vt[:],
                                         rhs=identb[:D, :D],
                                         is_transpose=True)
                        dest_of(t, tr2)
                        if post_tile is not None:
                            post_tile(t)

            def v_dest(t, tr2):
                p = 0
                while t >= TB[p + 1]:
                    p += 1
                off = (t - TB[p]) * P
                vsb = ep.tile([P, D], dt.bfloat16, tag="o")
                # v = dis_d * out1   (ghost lanes -> 0)
                nc.scalar.activation(out=vsb[:], in_=tr2[:], func=Copy,
                                     scale=disdt[:, t:t + 1])
                nc.sync.dma_start(out=v_shp[p][off:off + P, 0:D], in_=vsb[:])
                if DBG:
                    nc.sync.dma_start(out=dbg_v[t * P:(t + 1) * P, :],
                                      in_=vsb[:])

            def o_dest(t, tr2):
                vsb = ep.tile([P, D], dt.float32, tag="o2")
                nc.scalar.activation(out=vsb[:], in_=tr2[:], func=Copy)
                nc.sync.dma_start(out=out_sh[t * P:(t + 1) * P, :], in_=vsb[:])

            def fire_ag(t):
                for i in range(4):
                    if t == TB[i + 1] - 1:
                        lo, hi = int(REGION[i]), int(REGION[i + 1])
                        ag_insts[i] = nc.gpsimd.collective_compute(
                            "AllGather", mybir.AluOpType.bypass,
                            replica_groups=rg,
                            ins=[v_shp[i][:]], outs=[vfull[lo:hi, :]])

            layer(xup, w1t, b1t, True, v_dest, fire_ag)
            if DBG2:
                for i in range(0, NFULL, 1792):
                    hi = min(i + 1792, NFULL)
                    st = ep.tile([P, (1792 // P) * DP], dt.bfloat16, tag="dbg")
                    nc.sync.dma_start(out=st[:, :(hi - i) // P * DP],
                                      in_=vfull[i:hi, :])
                    nc.sync.dma_start(out=dbg_vf[i:hi, :],
                                      in_=st[:, :(hi - i) // P * DP])
            layer(vfull, w2t, b2t, False, o_dest,
                  gather_deps=lambda h: (ag_insts[0:2] if h == 0
                                         else ag_insts[2:4]))

    nc.compile()

    in_maps = []
    for k in range(C):
        in_maps.append({
            "xup": xu, "identp": ident_np,
            "identbp": identb_np,
            "idxp": S['idx_arr'][k], "disdp": S['disd_arr'][k],
            "w1p": W1.astype(ml_dtypes.bfloat16),
            "w2p": W2.astype(ml_dtypes.bfloat16),
            "b1p": b1.reshape(D, 1), "b2p": b2.reshape(D, 1),
        })
    global _compiled
    _compiled = (nc, in_maps)
    res = run_bass_kernel_spmd(nc, in_maps, list(range(C)))
    allout = np.stack([res.results[k]["out_sh"] for k in range(C)])
    out = allout[co, sl * 128 + la]
    return np.ascontiguousarray(out)


def profile_last():
    """Re-run the last compiled program with NTFF tracing; returns exec ns."""
    from concourse.bass_utils import run_bass_kernel_spmd
    assert _compiled is not None
    nc, in_maps = _compiled
    r = run_bass_kernel_spmd(nc, in_maps, list(range(C)), trace=True)
    return r.exec_time_ns
